# revision 13
# baseline (speedup 1.0000x reference)
"""Trainium2 Bass kernel for fixed-span (banded) multi-head attention.

Model (see reference): B=4, T=1024, F=512, H=8, DK=64, SPAN=100
    q,k,v = proj(x);  banded attention (query i attends keys [i-50, i+49]);
    out = attn_out @ Wo + bo.

Sharding: 8 cores = batch(4) x seq-half(2), fully data-parallel.  Each core
processes 512 queries of one batch with a 64-wide k/v halo on both sides
(640 kv positions), so the banded attention is entirely local.  Host gathers
the 8 (512, 512) outputs into the full (4, 1024, 512) result.

Device algorithm per core (all fp32):
  - PE-transpose x_q (512x512), x_k, x_v (640x512) into feature-major layout.
  - q^T/k^T via lhsT=W natural + rhs=x^T;  v token-major via lhsT=x_v^T,
    stored as v_ext [t, h, 65] with a ones column appended per head.
    Wq/bq are pre-scaled by 1/sqrt(DK) on the host.
  - Per query-block (128) x head-parity-group (4 heads): scores^T = k^T.T@q^T
    in [s, t] layout (2 kv chunks of 128; heads grouped by parity so every
    matmul into one PSUM bank shares its base partition - mixing row groups
    within a bank is fatal on HW).  exp with no max subtraction (scores are
    O(1) by construction), multiply by a host-built binary band mask.
  - AV: lhsT = p^T slice directly (no transposes), rhs = v_ext [s, 65]:
    one matmul emits both attn@v (cols 0-63) and the softmax denominator
    (col 64) in token-major layout; normalization is a per-partition-scalar
    multiply fused into the PSUM->SBUF evacuation.
  - x_att is PE-transposed back to feature-major for the output projection
    (lhsT = x_att^T, rhs = Wo natural) + bo, DMA out.
"""

import numpy as np

import concourse.bass as bass
import concourse.tile as tile
from concourse import bacc, mybir
from concourse.bass_utils import run_bass_kernel_spmd
from concourse.masks import make_identity

B, T, F = 4, 1024, 512
H, DK, SPAN = 8, 64, 100
PAD_L, PAD_R = 50, 49
TL = 512            # queries per core
HALO = 64
KVL = TL + 2 * HALO  # 640
NQB = TL // 128      # 4 query blocks
NFC = F // 128       # 4 feature chunks
NKVT = KVL // 128    # 5 kv token tiles
FP = mybir.dt.float32
FR = mybir.dt.float32r


def _build_nc(repeat: int = 1, f32r: bool = False) -> bacc.Bacc:
    nc = bacc.Bacc("TRN2", target_bir_lowering=False, debug=False, num_devices=8)

    xq_d = nc.dram_tensor("xq", [TL, F], FP, kind="ExternalInput").ap()
    xk_d = nc.dram_tensor("xk", [KVL, F], FP, kind="ExternalInput").ap()
    xv_d = nc.dram_tensor("xv", [KVL, F], FP, kind="ExternalInput").ap()
    WDT = FR if f32r else FP
    wq_d = nc.dram_tensor("wq", [F, F], WDT, kind="ExternalInput").ap()
    wk_d = nc.dram_tensor("wk", [F, F], WDT, kind="ExternalInput").ap()
    wv_d = nc.dram_tensor("wv", [F, F], WDT, kind="ExternalInput").ap()
    wo_d = nc.dram_tensor("wo", [F, F], WDT, kind="ExternalInput").ap()
    bq_d = nc.dram_tensor("bq", [F], FP, kind="ExternalInput").ap()
    bk_d = nc.dram_tensor("bk", [F], FP, kind="ExternalInput").ap()
    bv_d = nc.dram_tensor("bv", [F], FP, kind="ExternalInput").ap()
    bo_d = nc.dram_tensor("bo", [F], FP, kind="ExternalInput").ap()
    mt_d = nc.dram_tensor("maskt", [NKVT, 128, 256], FP, kind="ExternalInput").ap()
    out_d = nc.dram_tensor("out", [TL, F], FP, kind="ExternalOutput").ap()

    XDT = FR if f32r else FP

    with tile.TileContext(nc) as tc:
        with (
            tc.tile_pool(name="persist", bufs=1) as pp,
            tc.tile_pool(name="stage", bufs=4) as stage,
            tc.tile_pool(name="pt", bufs=8) as ptp,
            tc.tile_pool(name="rs", bufs=2) as rsp,
            tc.tile_pool(name="outs", bufs=2) as outp,
            tc.tile_pool(name="ps", bufs=4, space="PSUM") as psp,
        ):
            # persistent SBUF arrays ----------------------------------------------
            xqT = [pp.tile([128, TL], XDT, tag=f"xqT{fc}", name=f"xqT{fc}")
                   for fc in range(NFC)]
            xkT = [pp.tile([128, KVL], XDT, tag=f"xkT{fc}", name=f"xkT{fc}")
                   for fc in range(NFC)]
            xvT = [pp.tile([128, KVL], XDT, tag=f"xvT{fc}", name=f"xvT{fc}")
                   for fc in range(NFC)]
            qT = [pp.tile([128, TL], XDT, tag=f"qT{fc}", name=f"qT{fc}")
                  for fc in range(NFC)]
            kT = [pp.tile([128, KVL], XDT, tag=f"kT{fc}", name=f"kT{fc}")
                  for fc in range(NFC)]
            v_ext = [pp.tile([128, H, DK + 1], FP, tag=f"v{tt}", name=f"v{tt}")
                     for tt in range(NKVT)]
            xatt = [pp.tile([128, F], FP, tag=f"xatt{qb}", name=f"xatt{qb}")
                    for qb in range(NQB)]
            xattT = [pp.tile([128, TL], XDT, tag=f"xattT{fc}", name=f"xattT{fc}")
                     for fc in range(NFC)]

            ident = pp.tile([128, 128], FP, tag="ident")
            make_identity(nc, ident[:, :])

            def _emit():
                # ---- phase A: input DMAs (x first so transposes start early) ----
                def load_w(name, d):
                    tiles = []
                    for kc in range(NFC):
                        t = pp.tile([128, F], XDT, tag=f"{name}{kc}",
                                    name=f"{name}{kc}")
                        nc.sync.dma_start(out=t, in_=d[kc * 128:(kc + 1) * 128, :])
                        tiles.append(t)
                    return tiles

                def load_x(src, n_t):
                    tiles = []
                    for tt in range(n_t):
                        xt = stage.tile([128, F], FP, tag="xstage")
                        nc.sync.dma_start(out=xt,
                                          in_=src[tt * 128:(tt + 1) * 128, :])
                        tiles.append(xt)
                    return tiles

                w_sb = {}
                xq_t = load_x(xq_d, NFC)
                w_sb["wq"] = load_w("wq", wq_d)
                xk_t = load_x(xk_d, NKVT)
                w_sb["wk"] = load_w("wk", wk_d)
                xv_t = load_x(xv_d, NKVT)
                w_sb["wv"] = load_w("wv", wv_d)

                mt_sb = []
                for u in range(NKVT):
                    t = pp.tile([128, 256], FP, tag=f"mt{u}", name=f"mt{u}")
                    nc.sync.dma_start(out=t, in_=mt_d[u])
                    mt_sb.append(t)

                bq_sb = pp.tile([128, NFC], FP, tag="bq", name="bq_sb")
                nc.sync.dma_start(out=bq_sb,
                                  in_=bq_d.rearrange("(c p) -> p c", p=128))
                bk_sb = pp.tile([128, NFC], FP, tag="bk", name="bk_sb")
                nc.sync.dma_start(out=bk_sb,
                                  in_=bk_d.rearrange("(c p) -> p c", p=128))
                bv_bc = pp.tile([128, F], FP, tag="bv_bc", name="bv_bc")
                nc.sync.dma_start(
                    out=bv_bc,
                    in_=bass.AP(tensor=bv_d.tensor, offset=bv_d.offset,
                                ap=[[0, 128], [1, F]]))
                bo_bc = pp.tile([128, F], FP, tag="bo_bc", name="bo_bc")
                nc.sync.dma_start(
                    out=bo_bc,
                    in_=bass.AP(tensor=bo_d.tensor, offset=bo_d.offset,
                                ap=[[0, 128], [1, F]]))

                # ---- phase B: transpose inputs into feature-major ---------------
                for tiles, dstT in ((xq_t, xqT), (xk_t, xkT), (xv_t, xvT)):
                    for tt, xt in enumerate(tiles):
                        for fc in range(NFC):
                            ps = psp.tile([128, 128], FP, tag="ps")
                            nc.tensor.transpose(
                                ps, xt[:, fc * 128:(fc + 1) * 128], ident)
                            nc.scalar.copy(
                                out=dstT[fc][:, tt * 128:(tt + 1) * 128], in_=ps)

                w_sb["wo"] = load_w("wo", wo_d)

                # ---- phase C: projections ---------------------------------------
                for mc in range(NFC):
                    ps = psp.tile([128, TL], FP, tag="ps")
                    for kc in range(NFC):
                        nc.tensor.matmul(
                            ps, lhsT=w_sb["wq"][kc][:, mc * 128:(mc + 1) * 128],
                            rhs=xqT[kc], start=(kc == 0), stop=(kc == NFC - 1))
                    nc.vector.tensor_scalar_add(qT[mc], in0=ps,
                                                scalar1=bq_sb[:, mc:mc + 1])
                for mc in range(NFC):
                    for ns, nw in ((0, 320), (320, 320)):
                        ps = psp.tile([128, 320], FP, tag="ps")
                        for kc in range(NFC):
                            nc.tensor.matmul(
                                ps[:, :nw],
                                lhsT=w_sb["wk"][kc][:, mc * 128:(mc + 1) * 128],
                                rhs=xkT[kc][:, ns:ns + nw],
                                start=(kc == 0), stop=(kc == NFC - 1))
                        nc.vector.tensor_scalar_add(
                            kT[mc][:, ns:ns + nw], in0=ps[:, :nw],
                            scalar1=bk_sb[:, mc:mc + 1])
                # v_ext[t, h, 0:64] = (x_v @ Wv + bv)[t, h], v_ext[t, h, 64] = 1
                for tt in range(NKVT):
                    ps = psp.tile([128, F], FP, tag="ps")
                    for kc in range(NFC):
                        nc.tensor.matmul(
                            ps, lhsT=xvT[kc][:, tt * 128:(tt + 1) * 128],
                            rhs=w_sb["wv"][kc], start=(kc == 0),
                            stop=(kc == NFC - 1))
                    for h in range(H):
                        nc.vector.tensor_add(
                            v_ext[tt][:, h, 0:DK],
                            ps[:, h * DK:(h + 1) * DK],
                            bv_bc[:, h * DK:(h + 1) * DK])
                    nc.vector.memset(v_ext[tt][:, :, DK:DK + 1], 1.0)

                # ---- phase D: banded attention ----------------------------------
                # chunk-major: kv chunk u serves query blocks u-1 and u, so the
                # scores matmul has a 256-wide moving operand (f32r full rate).
                # heads grouped by parity: all 4 score matmuls in one PSUM bank
                # share their base partition (mixing row groups in a bank is
                # fatal on HW).
                pts2 = [[None, None] for _ in range(NKVT)]

                def emit_scores(u):
                    t0 = max(0, (u - 1) * 128)
                    t1 = min(TL, (u + 1) * 128)
                    w = t1 - t0
                    for hg in range(2):
                        r0 = hg * DK
                        sc = psp.tile([128, 4, 256], FP, tag="sc2", bufs=2)
                        for h4 in range(4):
                            nc.tensor.matmul(
                                sc[:, h4, 0:w],
                                lhsT=kT[h4][r0:r0 + DK, 128 * u:128 * u + 128],
                                rhs=qT[h4][r0:r0 + DK, t0:t1],
                                start=True, stop=True)
                        pt = ptp.tile([128, 4, 256], FP, tag="pt")
                        nc.scalar.activation(
                            pt[:, :, 0:w], sc[:, :, 0:w],
                            mybir.ActivationFunctionType.Exp)
                        m = mt_sb[u]
                        m_bc = bass.AP(tensor=m.tensor, offset=m.offset,
                                       ap=[m.ap[0], [0, 4], [1, w]])
                        nc.vector.tensor_mul(pt[:, :, 0:w], pt[:, :, 0:w], m_bc)
                        pts2[u][hg] = pt

                emit_scores(0)
                emit_scores(1)
                for u in range(1, NKVT):
                    if u + 1 < NKVT:
                        emit_scores(u + 1)
                    qb = u - 1
                    for hg in range(2):
                        av = psp.tile([128, 4, DK + 1], FP, tag="ps")
                        for h4 in range(4):
                            h = 2 * h4 + hg
                            for c in range(2):
                                uu = qb + c
                                off = qb * 128 - max(0, (uu - 1) * 128)
                                nc.tensor.matmul(
                                    av[:, h4, :],
                                    lhsT=pts2[uu][hg][:, h4, off:off + 128],
                                    rhs=v_ext[uu][:, h, :],
                                    start=(c == 0), stop=(c == 1))
                        rs = rsp.tile([128, 4, 1], FP, tag="rs")
                        nc.vector.reciprocal(rs, av[:, :, DK:DK + 1])
                        for h4 in range(4):
                            h = 2 * h4 + hg
                            nc.vector.tensor_scalar_mul(
                                xatt[qb][:, h * DK:(h + 1) * DK],
                                in0=av[:, h4, 0:DK], scalar1=rs[:, h4, :])

                # ---- phase E: transpose x_att, output projection ----------------
                for qb in range(NQB):
                    for fc in range(NFC):
                        ps = psp.tile([128, 128], FP, tag="ps")
                        nc.tensor.transpose(
                            ps, xatt[qb][:, fc * 128:(fc + 1) * 128], ident)
                        nc.scalar.copy(
                            out=xattT[fc][:, qb * 128:(qb + 1) * 128], in_=ps)
                for tt in range(NFC):
                    ps = psp.tile([128, F], FP, tag="ps")
                    for kc in range(NFC):
                        nc.tensor.matmul(
                            ps, lhsT=xattT[kc][:, tt * 128:(tt + 1) * 128],
                            rhs=w_sb["wo"][kc], start=(kc == 0),
                            stop=(kc == NFC - 1))
                    ot = outp.tile([128, F], FP, tag="ot")
                    nc.vector.tensor_add(ot, ps, bo_bc)
                    nc.sync.dma_start(out=out_d[tt * 128:(tt + 1) * 128, :],
                                      in_=ot)

            for _rep in range(repeat):
                _emit()

    nc.compile()
    return nc


_NC_CACHE = {}


def _get_nc(repeat: int = 1, f32r: bool = False):
    key = (repeat, f32r)
    if key not in _NC_CACHE:
        _NC_CACHE[key] = _build_nc(repeat, f32r)
    return _NC_CACHE[key]


def _core_in_map(inputs, core, w_host):
    b, half = core // 2, core % 2
    q0 = half * TL
    g0 = q0 - HALO
    xq = np.ascontiguousarray(inputs["query"][b, q0:q0 + TL], dtype=np.float32)
    xk = np.zeros((KVL, F), np.float32)
    xv = np.zeros((KVL, F), np.float32)
    lo, hi = max(0, g0), min(T, g0 + KVL)
    xk[lo - g0:hi - g0] = inputs["key"][b, lo:hi]
    xv[lo - g0:hi - g0] = inputs["value"][b, lo:hi]

    m = np.asarray(inputs["mask"][b, 0])
    s = np.arange(128)[:, None]
    maskt = np.zeros((NKVT, 128, 256), np.float32)
    for u in range(NKVT):
        t0 = max(0, (u - 1) * 128)
        t1 = min(TL, (u + 1) * 128)
        tl = np.arange(t1 - t0)[None, :]
        kv_g = g0 + 128 * u + s                 # (128, 1) global kv pos
        q_g = q0 + t0 + tl                      # (1, w) global query pos
        d = kv_g - q_g
        band = (d >= -PAD_L) & (d <= PAD_R)
        rng = (kv_g >= 0) & (kv_g < T)
        mk = np.where(rng, m[np.clip(kv_g, 0, T - 1)] != 0, False)
        maskt[u, :, 0:t1 - t0] = (band & rng & mk).astype(np.float32)

    return {"xq": xq, "xk": xk, "xv": xv, "maskt": maskt, **w_host}


def _w_host(inputs):
    scale = np.float32(1.0 / np.sqrt(DK))
    return {
        "wq": np.ascontiguousarray(inputs["Wq"], np.float32) * scale,
        "bq": np.ascontiguousarray(inputs["bq"], np.float32) * scale,
        "wk": np.ascontiguousarray(inputs["Wk"], np.float32),
        "bk": np.ascontiguousarray(inputs["bk"], np.float32),
        "wv": np.ascontiguousarray(inputs["Wv"], np.float32),
        "bv": np.ascontiguousarray(inputs["bv"], np.float32),
        "wo": np.ascontiguousarray(inputs["Wo"], np.float32),
        "bo": np.ascontiguousarray(inputs["bo"], np.float32),
    }


def kernel(**inputs) -> np.ndarray:
    nc = _get_nc(f32r=True)
    w_host = _w_host(inputs)
    in_maps = [_core_in_map(inputs, core, w_host) for core in range(8)]
    res = run_bass_kernel_spmd(nc, in_maps, core_ids=list(range(8)))
    out = np.zeros((B, T, F), np.float32)
    for core in range(8):
        b, half = core // 2, core % 2
        out[b, half * TL:(half + 1) * TL] = res.results[core]["out"]
    return out


if __name__ == "__main__":
    import reference as ref
    inputs = ref.setup_inputs()
    expected = np.asarray(ref.reference(**inputs))
    got = kernel(**{k: np.asarray(v) for k, v in inputs.items()})
    err = np.abs(got - expected).max()
    print("absmax:", err, "rel:", err / np.abs(expected).max())
